# revision 21
# baseline (speedup 1.0000x reference)
"""Grouped-query attention (B=2, T=2048, D=1024, 16 Q heads / 4 KV heads) on
8 Trainium2 NeuronCores — zero-collective version, v2 (pipelined).

Sharding: core i handles batch b = i//4 and head-group g = i%4 (query heads
4g..4g+3, KV head g).  Everything is computed in "transposed" layout
(features on partitions, tokens on the free axis).  bf16 operands with fp32
PSUM accumulation.

v2 changes vs v1 (both correct; v1 measured 205us on HW):
  * KV/Q projections run contraction-outer so the first matmul only waits
    for the first 512KB x^T chunk instead of the whole 4MB load.
  * The softmax normalization is decoupled from the PE critical path:
    oacc (PSUM) is copied out *unnormalized* right after the O-matmuls
    finish (fast DVE copies), the denominators are collected into a
    [16, 512] tile, inverted in one batched reciprocal_approx_fast call
    per q-block, broadcast on GPSIMD, and multiplied into cc on GPSIMD.
    PSUM banks recycle ~5us earlier per group, which keeps the PE busy
    and the HAM clock-gate warm (v1 lost ~60us to per-group PE stalls +
    the resulting 1.2GHz throttled matmuls).
  * out-proj matmuls for q-block qb are deferred into the middle of the
    (qb+1) attention groups so they never stall the PE on the normalize.
  * engine rebalance: Z-row copies and oacc copies on DVE, normalize
    multiplies on GPSIMD, exp stays on ACT (table preloaded at t=0).

No device collectives: each core writes its partial out^T (contraction over
its own 256 concat dims) and the HOST sums the 4 partials per batch.
"""

import sys

if "/opt/trn_rl_repo" not in sys.path:
    sys.path.insert(0, "/opt/trn_rl_repo")

import numpy as np

import concourse.bass as bass
import concourse.mybir as mybir
import concourse.tile as tile
from concourse import bacc, library_config
from concourse.bass import ds

F32 = mybir.dt.float32
BF16 = mybir.dt.bfloat16

B, T, D = 2, 2048, 1024
H, KVH, HD = 16, 4, 64
G = H // KVH            # 4 query heads per core
GD = G * HD             # 256 query dims per core
NCORES = 8
PB = 128                # partition block (s-chunk size)
QB = 512                # q block (matmul moving free dim)
NQB = T // QB           # 4
NSC = T // PB           # 16
NDC = D // PB           # 8 contraction chunks of D
EXP_GRP = 2             # s-chunk slots per exp() call (2 PSUM banks each)


def _mask_plan(mask2d):
    """Per q-block list of (j, avlo, mode, gix).

    mode 0: fully visible chunk.  mode 1: causal-diagonal chunk (columns
    below avlo are dead, the [avlo, avlo+128) square is multiplied by the
    triangular keep-mask).  mode 2: generic chunk, multiplied by keep-mask
    tile gix.  Chunks whose block is fully masked are dropped.
    """
    causal = np.array_equal(mask2d, np.triu(np.ones((T, T), dtype=bool), k=1))
    plan = []
    gen_tiles = []
    if causal:
        for qb in range(NQB):
            chunks = [(j, 0, 0, -1) for j in range(4 * qb)]
            chunks += [(4 * qb + k, PB * k, 1, -1) for k in range(4)]
            plan.append(chunks)
        kind = "causal"
    elif not mask2d.any():
        plan = [[(j, 0, 0, -1) for j in range(NSC)] for _ in range(NQB)]
        kind = "nomask"
    else:
        for qb in range(NQB):
            chunks = []
            for j in range(NSC):
                sub = mask2d[QB * qb:QB * (qb + 1), PB * j:PB * (j + 1)]
                if sub.all():
                    continue
                if not sub.any():
                    chunks.append((j, 0, 0, -1))
                else:
                    gen_tiles.append(np.ascontiguousarray((~sub).T))
                    chunks.append((j, 0, 2, len(gen_tiles) - 1))
            plan.append(chunks)
        kind = "generic"
    genmask = (
        np.stack(gen_tiles, axis=0).astype(np.float32)
        if gen_tiles else np.zeros((0, PB, QB), np.float32)
    )
    return kind, plan, genmask


def _build(plan, ngen, has_bias):
    nc = bacc.Bacc(
        "TRN2", target_bir_lowering=False, debug=False, num_devices=NCORES
    )

    xt_d = nc.dram_tensor("xt", [PB, NDC * T], BF16, kind="ExternalInput")
    wq_d = nc.dram_tensor("wq", [PB, NDC * GD], BF16, kind="ExternalInput")
    wkv_d = nc.dram_tensor("wkv", [PB, NDC * PB], BF16, kind="ExternalInput")
    wo_d = nc.dram_tensor("wo", [PB, 2 * D], BF16, kind="ExternalInput")
    tri_d = nc.dram_tensor("tri", [PB, PB], BF16, kind="ExternalInput")
    id_d = nc.dram_tensor("ident", [PB, HD], BF16, kind="ExternalInput")
    out_d = nc.dram_tensor("outT", [PB, NDC * T], BF16, kind="ExternalOutput")
    gen_d = None
    if ngen:
        gen_d = nc.dram_tensor("genmask", [ngen, PB, QB], BF16, kind="ExternalInput")
    if has_bias:
        bq_d = nc.dram_tensor("bqp", [PB, 2], F32, kind="ExternalInput")
        bkv_d = nc.dram_tensor("bkvp", [PB, 1], F32, kind="ExternalInput")

    NG = 2 * NQB            # number of attention groups (qb, p)

    with tile.TileContext(nc) as tc:
        with (
            tc.tile_pool(name="wts", bufs=1) as wpool,
            tc.tile_pool(name="qkv", bufs=1) as qkvpool,
            tc.tile_pool(name="pp", bufs=4) as ppool,
            tc.tile_pool(name="oo", bufs=2) as opool,
            tc.tile_pool(name="zz", bufs=4) as zpool,
        ):
            # ---- constant loads + ACT exp-table preload ----------------
            tri = wpool.tile([PB, PB], BF16, tag="tri", name="tri")
            nc.sync.dma_start(tri[:], tri_d[:])
            ident = wpool.tile([PB, HD], BF16, tag="ident", name="ident")
            nc.sync.dma_start(ident[:], id_d[:])
            nc.gpsimd.load_library(library_config.attnmlp)
            warm = wpool.tile([1, 4], F32, tag="warm", name="warm")
            nc.vector.memset(warm[:, 0:2], 0.0)
            nc.scalar.activation(
                warm[:, 2:4], warm[:, 0:2],
                mybir.ActivationFunctionType.Exp, scale=1.0,
            )

            # all inputs on ONE ring in need-order (the HWDGE engines are
            # shared between rings, so two active rings just interleave —
            # a single prioritized stream gets the first-needed tensors
            # to SBUF soonest): wkv, then x^T chunk by chunk, then wq,
            # then the rest.
            wkv_sb = wpool.tile([PB, NDC * PB], BF16, tag="wkv", name="wkv")
            nc.sync.dma_start(wkv_sb[:, 0:PB], wkv_d[:, 0:PB])
            xt = wpool.tile([PB, NDC * T], BF16, tag="xt", name="xt")
            # chunk 0 lands in quarters so the very first matmul (which
            # reads only [:, 0:512]) starts ~1.3us earlier
            for k4 in range(4):
                nc.sync.dma_start(
                    xt[:, ds(QB * k4, QB)], xt_d[:, ds(QB * k4, QB)]
                )
            nc.sync.dma_start(wkv_sb[:, PB:], wkv_d[:, PB:])
            for c in range(1, NDC):
                nc.sync.dma_start(xt[:, ds(T * c, T)], xt_d[:, ds(T * c, T)])
            wq_sb = wpool.tile([PB, NDC * GD], BF16, tag="wq", name="wq")
            for c in range(NDC):
                nc.sync.dma_start(wq_sb[:, ds(GD * c, GD)], wq_d[:, ds(GD * c, GD)])
            wo_sb = wpool.tile([PB, 2 * D], BF16, tag="wo", name="wo")
            nc.sync.dma_start(wo_sb[:], wo_d[:])
            if has_bias:
                bq_sb = wpool.tile([PB, 2], F32, tag="bq", name="bq")
                nc.sync.dma_start(bq_sb[:], bq_d[:])
                bkv_sb = wpool.tile([PB, 1], F32, tag="bkv", name="bkv")
                nc.sync.dma_start(bkv_sb[:], bkv_d[:])

            def psum_to_sbuf(dst, src, bias_col):
                if bias_col is not None:
                    nc.vector.tensor_scalar(
                        dst, src, scalar1=bias_col, scalar2=None,
                        op0=mybir.AluOpType.add,
                    )
                else:
                    nc.vector.tensor_copy(dst, src)

            # ---- projections (contraction-outer: first matmuls start as
            # ---- soon as x^T chunk 0 lands) ----------------------------
            qt = [
                qkvpool.tile([PB, T], BF16, tag="qt0", name="qt0"),
                qkvpool.tile([PB, T], BF16, tag="qt1", name="qt1"),
            ]
            kvt = qkvpool.tile([PB, T], BF16, tag="kvt", name="kvt")
            vt = qkvpool.tile([PB, NSC, HD + 1], BF16, tag="vt", name="vt")
            nc.vector.memset(vt[:, :, HD:HD + 1], 1.0)

            with tc.tile_pool(name="pps", bufs=1, space="PSUM") as pps:
                # KV projection, contraction-outer over 4 PSUM banks
                kv_ps = [
                    pps.tile([PB, QB], F32, tag="qps", bufs=4, name="kvps")
                    for _ in range(NQB)
                ]
                for c in range(NDC):
                    for nqi in range(NQB):
                        nc.tensor.matmul(
                            kv_ps[nqi][:],
                            wkv_sb[:, ds(PB * c, PB)],
                            xt[:, ds(T * c + QB * nqi, QB)],
                            start=(c == 0),
                            stop=(c == NDC - 1),
                        )
                for nqi in range(NQB):
                    psum_to_sbuf(
                        kvt[:, ds(QB * nqi, QB)], kv_ps[nqi][:],
                        bkv_sb[:, 0:1] if has_bias else None,
                    )

                # Q projection p=0 over 4 banks, then p=1 (reuses banks)
                for p in (0, 1):
                    q_ps = [
                        pps.tile([PB, QB], F32, tag="qps", bufs=4, name="qps")
                        for _ in range(NQB)
                    ]
                    for c in range(NDC):
                        for nqi in range(NQB):
                            nc.tensor.matmul(
                                q_ps[nqi][:],
                                wq_sb[:, ds(GD * c + PB * p, PB)],
                                xt[:, ds(T * c + QB * nqi, QB)],
                                start=(c == 0),
                                stop=(c == NDC - 1),
                            )
                    for nqi in range(NQB):
                        psum_to_sbuf(
                            qt[p][:, ds(QB * nqi, QB)], q_ps[nqi][:],
                            bq_sb[:, ds(p, 1)] if has_bias else None,
                        )
                    if p == 0:
                        # V natural tiles (PE transposes of V^T), then
                        # duplicate K^T into rows 64:128 of each block so
                        # head-pair S-matmuls read aligned partition
                        # ranges.  Issued here so the transposes sit on
                        # the PE queue behind Q p=0 (deps long ready).
                        for nqi in range(NQB):
                            v_ps = pps.tile(
                                [PB, 4, HD], BF16, tag="vps", bufs=2, name="vps"
                            )
                            for k4 in range(4):
                                j = 4 * nqi + k4
                                nc.tensor.transpose(
                                    v_ps[:, k4, :],
                                    kvt[HD:PB, ds(PB * j, PB)],
                                    ident[HD:PB, :],
                                )
                            nc.vector.tensor_copy(
                                vt[:, ds(4 * nqi, 4), 0:HD], v_ps[:]
                            )
                            nc.gpsimd.dma_start(
                                kvt[HD:PB, ds(QB * nqi, QB)],
                                kvt[0:HD, ds(QB * nqi, QB)],
                            )

            # ---- attention + interleaved out-proj ----------------------
            cc = [
                qkvpool.tile([PB, T], BF16, tag="cc0", name="cc0"),
                qkvpool.tile([PB, T], BF16, tag="cc1", name="cc1"),
            ]
            # unnormalized O^T + Z staging per (qb, p, h): row 0:64 = O^T,
            # row 64 = Z, in bf16 so the normalize multiplies run in the
            # DVE 4x mode.
            ustage = {}
            zqs = {}

            with (
                tc.tile_pool(name="spsum", bufs=2, space="PSUM") as spsum,
                tc.tile_pool(name="opsum", bufs=1, space="PSUM") as opsum,
            ):
                norm_a_q = []   # deferred: reciprocal + scatter + broadcast
                norm_b_q = []   # deferred: normalize multiplies
                outp_q = []     # deferred: out-proj matmuls
                zbmap = {}

                def make_outproj(qb):
                    def emit():
                        for pt in range(NDC):
                            o_ps = opsum.tile(
                                [PB, QB], F32, tag="psb", bufs=2, name="psb"
                            )
                            for c2 in (0, 1):
                                nc.tensor.matmul(
                                    o_ps[:],
                                    wo_sb[:, ds(D * c2 + PB * pt, PB)],
                                    cc[c2][:, ds(QB * qb, QB)],
                                    start=(c2 == 0),
                                    stop=(c2 == 1),
                                )
                            ob = opool.tile(
                                [PB, QB], BF16, tag="ob", bufs=8, name="ob"
                            )
                            nc.vector.tensor_copy(ob[:], o_ps[:])
                            nc.sync.dma_start(
                                out_d[:, ds(T * pt + QB * qb, QB)], ob[:]
                            )
                    return emit

                def make_norm_a(qb, ps=(0, 1)):
                    def emit():
                        # batched reciprocal for the gathered Z rows, bf16
                        # cast, scatter to partition-0 rows, GPSIMD
                        # broadcast.  Row j of the gather tile is
                        # 2*(p - ps[0]) + h (partition slices must start
                        # at 0).
                        zq = zqs[(qb, ps)]
                        nj = 2 * len(ps)
                        zqi = zpool.tile([4, QB], F32, tag="zqi", bufs=2,
                                         name="zqi")
                        nc.vector.reciprocal(
                            zqi[ds(0, nj), :], zq[ds(0, nj), :]
                        )
                        zqb = zpool.tile([4, QB], BF16, tag="zqb", bufs=2,
                                         name="zqb")
                        nc.vector.tensor_copy(
                            zqb[ds(0, nj), :], zqi[ds(0, nj), :]
                        )
                        for p in ps:
                            for h in (0, 1):
                                j = 2 * (p - ps[0]) + h
                                zr = zpool.tile([1, QB], BF16, tag="zr",
                                                bufs=8, name="zr")
                                nc.gpsimd.dma_start(zr[:], zqb[j:j + 1, :])
                                zb = zpool.tile([HD, QB], BF16, tag="zb",
                                                bufs=8, name="zb")
                                nc.gpsimd.partition_broadcast(zb[:], zr[:])
                                zbmap[(qb, p, h)] = zb
                    return emit

                def make_norm_b(qb, ps=(0, 1)):
                    def emit():
                        for p in ps:
                            for h in (0, 1):
                                u = ustage.pop((qb, p, h))
                                zb = zbmap.pop((qb, p, h))
                                nc.vector.tensor_mul(
                                    cc[p][ds(HD * h, HD), ds(QB * qb, QB)],
                                    u[0:HD, :],
                                    zb[:],
                                )
                    return emit

                pending = None     # (grp, pg, oacc, qb, p, state) for grp_tail

                def grp_tail(grp, pg, oacc, qb, p, n_of, total):
                    # tri/generic masking (DVE) + O-matmuls (PE); when a
                    # head's accumulation completes, evacuate the whole
                    # [65, QB] PSUM tile (O^T + Z row) to SBUF in one fast
                    # copy so the bank recycles immediately.
                    for idx, (h, (j, avlo, mode, gix)) in enumerate(grp):
                        if mode == 1:
                            nc.vector.tensor_mul(
                                pg[:, idx, ds(avlo, PB)],
                                pg[:, idx, ds(avlo, PB)],
                                tri[:],
                            )
                        elif mode == 2:
                            gm = ppool.tile(
                                [PB, QB], BF16, tag="gm", bufs=4, name="gm"
                            )
                            nc.sync.dma_start(gm[:], gen_d[gix])
                            nc.vector.tensor_mul(
                                pg[:, idx, :], pg[:, idx, :], gm[:]
                            )
                        nc.tensor.matmul(
                            oacc[h][:, ds(avlo, QB - avlo)],
                            vt[:, j, :],
                            pg[:, idx, ds(avlo, QB - avlo)],
                            start=(n_of[h] == 0),
                            stop=(n_of[h] == total - 1),
                        )
                        n_of[h] += 1
                        if n_of[h] == total:
                            u = zpool.tile([HD + 1, QB], BF16, tag="ustg",
                                           bufs=6, name="ustg")
                            nc.vector.tensor_copy(u[:], oacc[h][:])
                            ustage[(qb, p, h)] = u
                            # eager Z-row gather on the (idle) GPSIMD ring;
                            # the last q-block normalizes per-p so it
                            # gathers into per-p tiles (rows must start at
                            # partition 0)
                            ps = ((p,) if qb == NQB - 1 else (0, 1))
                            if (qb, ps) not in zqs:
                                zqs[(qb, ps)] = zpool.tile(
                                    [4, QB], BF16, tag="zq", bufs=3, name="zq"
                                )
                            j = 2 * (p - ps[0]) + h
                            nc.gpsimd.dma_start(
                                zqs[(qb, ps)][j:j + 1, :],
                                u[HD:HD + 1, :],
                            )

                for qb in range(NQB):
                    chunks = plan[qb]
                    for p in (0, 1):
                        slots = [(h, ch) for ch in chunks for h in (0, 1)]
                        oacc = [
                            opsum.tile([HD + 1, QB], F32, tag="oacc", bufs=2,
                                       name="oacc")
                            for _ in (0, 1)
                        ]
                        n_of = {0: 0, 1: 0}
                        total = len(chunks)
                        npairs = 0
                        for gi0 in range(0, len(slots), EXP_GRP):
                            grp = slots[gi0:gi0 + EXP_GRP]
                            avg = grp[0][1][1]
                            sg = spsum.tile(
                                [PB, EXP_GRP, QB], F32, tag="sg", bufs=2,
                                name="sg"
                            )
                            pg = ppool.tile(
                                [PB, EXP_GRP, QB], BF16, tag="pg", bufs=6,
                                name="pg"
                            )
                            for idx, (h, (j, avlo, mode, gix)) in enumerate(grp):
                                nc.tensor.matmul(
                                    sg[:, idx, ds(avlo, QB - avlo)],
                                    kvt[ds(HD * h, HD), ds(PB * j, PB)],
                                    qt[p][ds(HD * h, HD), ds(QB * qb + avlo, QB - avlo)],
                                    start=True,
                                    stop=True,
                                )
                            if pending is not None:
                                grp_tail(*pending)
                                pending = None
                            nc.scalar.activation(
                                pg[:, 0:len(grp), ds(avg, QB - avg)],
                                sg[:, 0:len(grp), ds(avg, QB - avg)],
                                mybir.ActivationFunctionType.Exp,
                                scale=1.0 / float(np.sqrt(HD)),
                            )
                            pending = (grp, pg, oacc, qb, p, n_of, total)
                            npairs += 1
                            # staged deferral: the reciprocal/broadcast for
                            # the previous q-block a couple of pairs in, the
                            # normalize multiplies several pairs later (the
                            # GPSIMD broadcast chain is ~6us), the out-proj
                            # matmuls a couple of pairs into the NEXT (p=1)
                            # group.  The last q-block's p=0 half
                            # normalizes early inside (3, p=1) so only the
                            # p=1 half remains for the epilogue.
                            if p == 0:
                                if npairs == 2 and norm_a_q:
                                    norm_a_q.pop(0)()
                                if npairs == 8 and norm_b_q:
                                    norm_b_q.pop(0)()
                            else:
                                if npairs == 2 and outp_q:
                                    outp_q.pop(0)()
                                if qb == NQB - 1:
                                    if npairs == 4:
                                        make_norm_a(qb, (0,))()
                                    if npairs == 10:
                                        make_norm_b(qb, (0,))()
                        if p == 1 and qb < NQB - 1:
                            norm_a_q.append(make_norm_a(qb))
                            norm_b_q.append(make_norm_b(qb))
                            outp_q.append(make_outproj(qb))
                # epilogue: flush the last group's tail, then normalize the
                # last q-block's p=1 half and emit its out-proj
                if pending is not None:
                    grp_tail(*pending)
                    pending = None
                for q in (norm_a_q, norm_b_q):
                    for fn in q:
                        fn()
                make_norm_a(NQB - 1, (1,))()
                make_norm_b(NQB - 1, (1,))()
                for fn in outp_q:
                    fn()
                make_outproj(NQB - 1)()

    nc.compile()
    return nc


_CACHE = {}


def _get_program(mask2d, has_bias):
    kind, plan, genmask = _mask_plan(mask2d)
    if kind == "generic":
        key = ("generic", mask2d.tobytes(), has_bias)
    else:
        key = (kind, has_bias)
    if key not in _CACHE:
        _CACHE[key] = (_build(plan, len(genmask), has_bias), genmask)
    return _CACHE[key]


def _chunk_major(a, pb=PB):
    """[R, C] (R = k*pb) -> [pb, k*C] laid out chunk-major."""
    r, c = a.shape
    k = r // pb
    return np.ascontiguousarray(
        a.reshape(k, pb, c).transpose(1, 0, 2).reshape(pb, k * c)
    )


def _make_in_maps(x, mask2d, Wq, bq, Wk, bk, Wv, bv, Wo, bo, genmask, has_bias):
    tri = np.triu(np.ones((PB, PB), dtype=np.float32))
    id64 = np.concatenate(
        [np.zeros((HD, HD), np.float32), np.eye(HD, dtype=np.float32)], axis=0
    )

    bf = mybir.dt.np(BF16)

    in_maps = []
    for i in range(NCORES):
        b, g = divmod(i, 4)
        xbT = np.ascontiguousarray(x[b].T)
        wkv = np.concatenate(
            [Wk[:, HD * g:HD * (g + 1)], Wv[:, HD * g:HD * (g + 1)]], axis=1
        )
        m = {
            "xt": _chunk_major(xbT).astype(bf),
            "wq": _chunk_major(Wq[:, GD * g:GD * (g + 1)]).astype(bf),
            "wkv": _chunk_major(wkv).astype(bf),
            "wo": _chunk_major(Wo[GD * g:GD * (g + 1), :]).astype(bf),
            "tri": tri.astype(bf),
            "ident": id64.astype(bf),
        }
        if len(genmask):
            m["genmask"] = genmask.astype(bf)
        if has_bias:
            bq_g = bq[GD * g:GD * (g + 1)]
            m["bqp"] = np.ascontiguousarray(bq_g.reshape(2, PB).T).astype(np.float32)
            m["bkvp"] = np.concatenate(
                [bk[HD * g:HD * (g + 1)], bv[HD * g:HD * (g + 1)]]
            ).reshape(PB, 1).astype(np.float32)
        in_maps.append(m)
    return in_maps


def _assemble(results, bo):
    out = np.empty((B, T, D), dtype=np.float32)
    for b in range(B):
        acc = None
        for g in range(4):
            r = results[4 * b + g]["outT"]          # [128, 8*2048]
            partial = (
                r.astype(np.float32).reshape(PB, NDC, T).transpose(1, 0, 2).reshape(D, T)
            )
            acc = partial if acc is None else acc + partial
        out[b] = acc.T
    if bo is not None:
        out += bo
    return out


def run(inputs, trace=False):
    from concourse.bass_utils import run_bass_kernel_spmd

    x = np.asarray(inputs["x"], dtype=np.float32)
    mask2d = np.asarray(inputs["mask"]).reshape(T, T).astype(bool)
    Wq = np.asarray(inputs["Wq"], np.float32)
    bq = np.asarray(inputs["bq"], np.float32)
    Wk = np.asarray(inputs["Wk"], np.float32)
    bk = np.asarray(inputs["bk"], np.float32)
    Wv = np.asarray(inputs["Wv"], np.float32)
    bv = np.asarray(inputs["bv"], np.float32)
    Wo = np.asarray(inputs["Wo"], np.float32)
    bo = np.asarray(inputs["bo"], np.float32)
    has_bias = bool(bq.any() or bk.any() or bv.any())
    nc, genmask = _get_program(mask2d, has_bias)
    in_maps = _make_in_maps(
        x, mask2d, Wq, bq, Wk, bk, Wv, bv, Wo, bo, genmask, has_bias
    )
    res = run_bass_kernel_spmd(
        nc, in_maps, core_ids=list(range(NCORES)), trace=trace
    )
    return _assemble(res.results, bo if bo.any() else None), res


def kernel(**inputs) -> np.ndarray:
    out, _ = run(inputs, trace=False)
    return out


# revision 24
# speedup vs baseline: 1.0210x; 1.0210x over previous
"""Grouped-query attention (B=2, T=2048, D=1024, 16 Q heads / 4 KV heads) on
8 Trainium2 NeuronCores — zero-collective version, v2 (pipelined).

Sharding: core i handles batch b = i//4 and head-group g = i%4 (query heads
4g..4g+3, KV head g).  Everything is computed in "transposed" layout
(features on partitions, tokens on the free axis).  bf16 operands with fp32
PSUM accumulation.

v2 changes vs v1 (both correct; v1 measured 205us on HW):
  * KV/Q projections run contraction-outer so the first matmul only waits
    for the first 512KB x^T chunk instead of the whole 4MB load.
  * The softmax normalization is decoupled from the PE critical path:
    oacc (PSUM) is copied out *unnormalized* right after the O-matmuls
    finish (fast DVE copies), the denominators are collected into a
    [16, 512] tile, inverted in one batched reciprocal_approx_fast call
    per q-block, broadcast on GPSIMD, and multiplied into cc on GPSIMD.
    PSUM banks recycle ~5us earlier per group, which keeps the PE busy
    and the HAM clock-gate warm (v1 lost ~60us to per-group PE stalls +
    the resulting 1.2GHz throttled matmuls).
  * out-proj matmuls for q-block qb are deferred into the middle of the
    (qb+1) attention groups so they never stall the PE on the normalize.
  * engine rebalance: Z-row copies and oacc copies on DVE, normalize
    multiplies on GPSIMD, exp stays on ACT (table preloaded at t=0).

No device collectives: each core writes its partial out^T (contraction over
its own 256 concat dims) and the HOST sums the 4 partials per batch.
"""

import sys

if "/opt/trn_rl_repo" not in sys.path:
    sys.path.insert(0, "/opt/trn_rl_repo")

import numpy as np

import concourse.bass as bass
import concourse.mybir as mybir
import concourse.tile as tile
from concourse import bacc, library_config
from concourse.bass import ds

F32 = mybir.dt.float32
BF16 = mybir.dt.bfloat16

B, T, D = 2, 2048, 1024
H, KVH, HD = 16, 4, 64
G = H // KVH            # 4 query heads per core
GD = G * HD             # 256 query dims per core
NCORES = 8
PB = 128                # partition block (s-chunk size)
QB = 512                # q block (matmul moving free dim)
NQB = T // QB           # 4
NSC = T // PB           # 16
NDC = D // PB           # 8 contraction chunks of D
EXP_GRP = 2             # s-chunk slots per exp() call (2 PSUM banks each)


def _mask_plan(mask2d):
    """Per q-block list of (j, avlo, mode, gix).

    mode 0: fully visible chunk.  mode 1: causal-diagonal chunk (columns
    below avlo are dead, the [avlo, avlo+128) square is multiplied by the
    triangular keep-mask).  mode 2: generic chunk, multiplied by keep-mask
    tile gix.  Chunks whose block is fully masked are dropped.
    """
    causal = np.array_equal(mask2d, np.triu(np.ones((T, T), dtype=bool), k=1))
    plan = []
    gen_tiles = []
    if causal:
        for qb in range(NQB):
            chunks = [(j, 0, 0, -1) for j in range(4 * qb)]
            chunks += [(4 * qb + k, PB * k, 1, -1) for k in range(4)]
            plan.append(chunks)
        kind = "causal"
    elif not mask2d.any():
        plan = [[(j, 0, 0, -1) for j in range(NSC)] for _ in range(NQB)]
        kind = "nomask"
    else:
        for qb in range(NQB):
            chunks = []
            for j in range(NSC):
                sub = mask2d[QB * qb:QB * (qb + 1), PB * j:PB * (j + 1)]
                if sub.all():
                    continue
                if not sub.any():
                    chunks.append((j, 0, 0, -1))
                else:
                    gen_tiles.append(np.ascontiguousarray((~sub).T))
                    chunks.append((j, 0, 2, len(gen_tiles) - 1))
            plan.append(chunks)
        kind = "generic"
    genmask = (
        np.stack(gen_tiles, axis=0).astype(np.float32)
        if gen_tiles else np.zeros((0, PB, QB), np.float32)
    )
    return kind, plan, genmask


def _build(plan, ngen, has_bias):
    nc = bacc.Bacc(
        "TRN2", target_bir_lowering=False, debug=False, num_devices=NCORES
    )

    xt_d = nc.dram_tensor("xt", [PB, NDC * T], BF16, kind="ExternalInput")
    wq_d = nc.dram_tensor("wq", [PB, NDC * GD], BF16, kind="ExternalInput")
    wkv_d = nc.dram_tensor("wkv", [PB, NDC * PB], BF16, kind="ExternalInput")
    wo_d = nc.dram_tensor("wo", [PB, 2 * D], BF16, kind="ExternalInput")
    tri_d = nc.dram_tensor("tri", [PB, PB], BF16, kind="ExternalInput")
    id_d = nc.dram_tensor("ident", [PB, HD], BF16, kind="ExternalInput")
    out_d = nc.dram_tensor("outT", [PB, NDC * T], BF16, kind="ExternalOutput")
    gen_d = None
    if ngen:
        gen_d = nc.dram_tensor("genmask", [ngen, PB, QB], BF16, kind="ExternalInput")
    if has_bias:
        bq_d = nc.dram_tensor("bqp", [PB, 2], F32, kind="ExternalInput")
        bkv_d = nc.dram_tensor("bkvp", [PB, 1], F32, kind="ExternalInput")

    NG = 2 * NQB            # number of attention groups (qb, p)

    with tile.TileContext(nc) as tc:
        with (
            tc.tile_pool(name="wts", bufs=1) as wpool,
            tc.tile_pool(name="qkv", bufs=1) as qkvpool,
            tc.tile_pool(name="pp", bufs=4) as ppool,
            tc.tile_pool(name="oo", bufs=2) as opool,
            tc.tile_pool(name="zz", bufs=4) as zpool,
        ):
            # ---- constant loads + ACT exp-table preload ----------------
            tri = wpool.tile([PB, PB], BF16, tag="tri", name="tri")
            nc.sync.dma_start(tri[:], tri_d[:])
            ident = wpool.tile([PB, HD], BF16, tag="ident", name="ident")
            nc.sync.dma_start(ident[:], id_d[:])
            ones64 = wpool.tile([1, HD], BF16, tag="ones64", name="ones64")
            nc.vector.memset(ones64[:], 1.0)
            nc.gpsimd.load_library(library_config.attnmlp)
            warm = wpool.tile([1, 4], F32, tag="warm", name="warm")
            nc.vector.memset(warm[:, 0:2], 0.0)
            nc.scalar.activation(
                warm[:, 2:4], warm[:, 0:2],
                mybir.ActivationFunctionType.Exp, scale=1.0,
            )

            # all inputs on ONE ring in need-order (the HWDGE engines are
            # shared between rings, so two active rings just interleave —
            # a single prioritized stream gets the first-needed tensors
            # to SBUF soonest): wkv, then x^T chunk by chunk, then wq,
            # then the rest.
            wkv_sb = wpool.tile([PB, NDC * PB], BF16, tag="wkv", name="wkv")
            nc.sync.dma_start(wkv_sb[:, 0:PB], wkv_d[:, 0:PB])
            xt = wpool.tile([PB, NDC * T], BF16, tag="xt", name="xt")
            # chunk 0 lands in quarters so the very first matmul (which
            # reads only [:, 0:512]) starts ~1.3us earlier
            for k4 in range(4):
                nc.sync.dma_start(
                    xt[:, ds(QB * k4, QB)], xt_d[:, ds(QB * k4, QB)]
                )
            nc.sync.dma_start(wkv_sb[:, PB:], wkv_d[:, PB:])
            for c in range(1, NDC):
                nc.sync.dma_start(xt[:, ds(T * c, T)], xt_d[:, ds(T * c, T)])
            wq_sb = wpool.tile([PB, NDC * GD], BF16, tag="wq", name="wq")
            for c in range(NDC):
                nc.sync.dma_start(wq_sb[:, ds(GD * c, GD)], wq_d[:, ds(GD * c, GD)])
            wo_sb = wpool.tile([PB, 2 * D], BF16, tag="wo", name="wo")
            nc.sync.dma_start(wo_sb[:], wo_d[:])
            if has_bias:
                bq_sb = wpool.tile([PB, 2], F32, tag="bq", name="bq")
                nc.sync.dma_start(bq_sb[:], bq_d[:])
                bkv_sb = wpool.tile([PB, 1], F32, tag="bkv", name="bkv")
                nc.sync.dma_start(bkv_sb[:], bkv_d[:])

            def psum_to_sbuf(dst, src, bias_col):
                if bias_col is not None:
                    nc.vector.tensor_scalar(
                        dst, src, scalar1=bias_col, scalar2=None,
                        op0=mybir.AluOpType.add,
                    )
                else:
                    nc.vector.tensor_copy(dst, src)

            # ---- projections (contraction-outer: first matmuls start as
            # ---- soon as x^T chunk 0 lands) ----------------------------
            qt = [
                qkvpool.tile([PB, T], BF16, tag="qt0", name="qt0"),
                qkvpool.tile([PB, T], BF16, tag="qt1", name="qt1"),
            ]
            kvt = qkvpool.tile([PB, T], BF16, tag="kvt", name="kvt")
            vt = qkvpool.tile([PB, NSC, HD + 1], BF16, tag="vt", name="vt")
            nc.vector.memset(vt[:, :, HD:HD + 1], 1.0)

            with tc.tile_pool(name="pps", bufs=1, space="PSUM") as pps:
                # KV projection, contraction-outer over 4 PSUM banks
                kv_ps = [
                    pps.tile([PB, QB], F32, tag="qps", bufs=4, name="kvps")
                    for _ in range(NQB)
                ]
                for c in range(NDC):
                    for nqi in range(NQB):
                        nc.tensor.matmul(
                            kv_ps[nqi][:],
                            wkv_sb[:, ds(PB * c, PB)],
                            xt[:, ds(T * c + QB * nqi, QB)],
                            start=(c == 0),
                            stop=(c == NDC - 1),
                        )
                for nqi in range(NQB):
                    psum_to_sbuf(
                        kvt[:, ds(QB * nqi, QB)], kv_ps[nqi][:],
                        bkv_sb[:, 0:1] if has_bias else None,
                    )

                # Q projection p=0 over 4 banks, then p=1 (reuses banks)
                for p in (0, 1):
                    q_ps = [
                        pps.tile([PB, QB], F32, tag="qps", bufs=4, name="qps")
                        for _ in range(NQB)
                    ]
                    for c in range(NDC):
                        for nqi in range(NQB):
                            nc.tensor.matmul(
                                q_ps[nqi][:],
                                wq_sb[:, ds(GD * c + PB * p, PB)],
                                xt[:, ds(T * c + QB * nqi, QB)],
                                start=(c == 0),
                                stop=(c == NDC - 1),
                            )
                    for nqi in range(NQB):
                        psum_to_sbuf(
                            qt[p][:, ds(QB * nqi, QB)], q_ps[nqi][:],
                            bq_sb[:, ds(p, 1)] if has_bias else None,
                        )
                    if p == 0:
                        # V natural tiles (PE transposes of V^T), then
                        # duplicate K^T into rows 64:128 of each block so
                        # head-pair S-matmuls read aligned partition
                        # ranges.  Issued here so the transposes sit on
                        # the PE queue behind Q p=0 (deps long ready).
                        for nqi in range(NQB):
                            v_ps = pps.tile(
                                [PB, 4, HD], BF16, tag="vps", bufs=2, name="vps"
                            )
                            for k4 in range(4):
                                j = 4 * nqi + k4
                                nc.tensor.transpose(
                                    v_ps[:, k4, :],
                                    kvt[HD:PB, ds(PB * j, PB)],
                                    ident[HD:PB, :],
                                )
                            nc.vector.tensor_copy(
                                vt[:, ds(4 * nqi, 4), 0:HD], v_ps[:]
                            )
                            nc.gpsimd.dma_start(
                                kvt[HD:PB, ds(QB * nqi, QB)],
                                kvt[0:HD, ds(QB * nqi, QB)],
                            )

            # ---- attention + interleaved out-proj ----------------------
            cc = [
                qkvpool.tile([PB, T], BF16, tag="cc0", name="cc0"),
                qkvpool.tile([PB, T], BF16, tag="cc1", name="cc1"),
            ]
            # unnormalized O^T + Z staging per (qb, p, h): row 0:64 = O^T,
            # row 64 = Z, in bf16 so the normalize multiplies run in the
            # DVE 4x mode.
            ustage = {}
            zqs = {}

            with (
                tc.tile_pool(name="spsum", bufs=2, space="PSUM") as spsum,
                tc.tile_pool(name="opsum", bufs=1, space="PSUM") as opsum,
            ):
                norm_a_q = []   # deferred: reciprocal + scatter + broadcast
                norm_b_q = []   # deferred: normalize multiplies
                outp_q = []     # deferred: out-proj matmuls
                zbmap = {}

                def make_outproj(qb):
                    def emit():
                        for pt in range(NDC):
                            o_ps = opsum.tile(
                                [PB, QB], F32, tag="psb", bufs=2, name="psb"
                            )
                            for c2 in (0, 1):
                                nc.tensor.matmul(
                                    o_ps[:],
                                    wo_sb[:, ds(D * c2 + PB * pt, PB)],
                                    cc[c2][:, ds(QB * qb, QB)],
                                    start=(c2 == 0),
                                    stop=(c2 == 1),
                                )
                            ob = opool.tile(
                                [PB, QB], BF16, tag="ob", bufs=8, name="ob"
                            )
                            nc.vector.tensor_copy(ob[:], o_ps[:])
                            nc.sync.dma_start(
                                out_d[:, ds(T * pt + QB * qb, QB)], ob[:]
                            )
                    return emit

                def inv_z(qb, ps):
                    # 1/Z on the ACT engine as exp(-ln Z): both functions
                    # live in the same activation table as the attention
                    # exp, so no table reloads and the DVE FIFO (which
                    # carries the latency-critical masks) stays clear.
                    zq = zqs[(qb, ps)]
                    nj = 2 * len(ps)
                    zln = zpool.tile([4, QB], F32, tag="zln", bufs=2,
                                     name="zln")
                    nc.scalar.activation(
                        zln[ds(0, nj), :], zq[ds(0, nj), :],
                        mybir.ActivationFunctionType.Ln,
                    )
                    zqb = zpool.tile([4, QB], BF16, tag="zqb", bufs=2,
                                     name="zqb")
                    nc.scalar.activation(
                        zqb[ds(0, nj), :], zln[ds(0, nj), :],
                        mybir.ActivationFunctionType.Exp, scale=-1.0,
                    )
                    return zqb

                def make_norm_a(qb, ps=(0, 1)):
                    def emit():
                        # row j of the gather tile is 2*(p - ps[0]) + h
                        # (partition slices must start at 0)
                        zqb = inv_z(qb, ps)
                        for p in ps:
                            for h in (0, 1):
                                j = 2 * (p - ps[0]) + h
                                if j == 0:
                                    src = zqb[0:1, :]
                                else:
                                    zr = zpool.tile([1, QB], BF16, tag="zr",
                                                    bufs=8, name="zr")
                                    nc.gpsimd.dma_start(zr[:], zqb[j:j + 1, :])
                                    src = zr[:]
                                zb = zpool.tile([HD, QB], BF16, tag="zb",
                                                bufs=8, name="zb")
                                nc.gpsimd.partition_broadcast(zb[:], src)
                                zbmap[(qb, p, h)] = zb
                    return emit

                def make_norm_b(qb, ps=(0, 1)):
                    def emit():
                        for p in ps:
                            for h in (0, 1):
                                u = ustage.pop((qb, p, h))
                                zb = zbmap.pop((qb, p, h))
                                nc.vector.tensor_mul(
                                    cc[p][ds(HD * h, HD), ds(QB * qb, QB)],
                                    u[0:HD, :],
                                    zb[:],
                                )
                    return emit

                pending = None     # (grp, pg, oacc, qb, p, state) for grp_tail

                def grp_tail(grp, pg, oacc, qb, p, n_of, total):
                    # tri/generic masking (DVE) + O-matmuls (PE); when a
                    # head's accumulation completes, evacuate the whole
                    # [65, QB] PSUM tile (O^T + Z row) to SBUF in one fast
                    # copy so the bank recycles immediately.
                    for idx, (h, (j, avlo, mode, gix)) in enumerate(grp):
                        if mode == 1:
                            nc.vector.tensor_mul(
                                pg[:, idx, ds(avlo, PB)],
                                pg[:, idx, ds(avlo, PB)],
                                tri[:],
                            )
                        elif mode == 2:
                            gm = ppool.tile(
                                [PB, QB], BF16, tag="gm", bufs=4, name="gm"
                            )
                            nc.sync.dma_start(gm[:], gen_d[gix])
                            nc.vector.tensor_mul(
                                pg[:, idx, :], pg[:, idx, :], gm[:]
                            )
                        nc.tensor.matmul(
                            oacc[h][:, ds(avlo, QB - avlo)],
                            vt[:, j, :],
                            pg[:, idx, ds(avlo, QB - avlo)],
                            start=(n_of[h] == 0),
                            stop=(n_of[h] == total - 1),
                        )
                        n_of[h] += 1
                        if n_of[h] == total:
                            u = zpool.tile([HD + 1, QB], BF16, tag="ustg",
                                           bufs=6, name="ustg")
                            nc.vector.tensor_copy(u[:], oacc[h][:])
                            ustage[(qb, p, h)] = u
                            # eager Z-row gather on the (idle) GPSIMD ring;
                            # the last q-block normalizes per-p so it
                            # gathers into per-p tiles (rows must start at
                            # partition 0)
                            ps = ((p,) if qb == NQB - 1 else (0, 1))
                            if (qb, ps) not in zqs:
                                zqs[(qb, ps)] = zpool.tile(
                                    [4, QB], BF16, tag="zq", bufs=3, name="zq"
                                )
                            j = 2 * (p - ps[0]) + h
                            nc.gpsimd.dma_start(
                                zqs[(qb, ps)][j:j + 1, :],
                                u[HD:HD + 1, :],
                            )

                for qb in range(NQB):
                    chunks = plan[qb]
                    for p in (0, 1):
                        slots = [(h, ch) for ch in chunks for h in (0, 1)]
                        oacc = [
                            opsum.tile([HD + 1, QB], F32, tag="oacc", bufs=2,
                                       name="oacc")
                            for _ in (0, 1)
                        ]
                        n_of = {0: 0, 1: 0}
                        total = len(chunks)
                        npairs = 0
                        for gi0 in range(0, len(slots), EXP_GRP):
                            grp = slots[gi0:gi0 + EXP_GRP]
                            avg = grp[0][1][1]
                            sg = spsum.tile(
                                [PB, EXP_GRP, QB], F32, tag="sg", bufs=2,
                                name="sg"
                            )
                            pg = ppool.tile(
                                [PB, EXP_GRP, QB], BF16, tag="pg", bufs=6,
                                name="pg"
                            )
                            for idx, (h, (j, avlo, mode, gix)) in enumerate(grp):
                                nc.tensor.matmul(
                                    sg[:, idx, ds(avlo, QB - avlo)],
                                    kvt[ds(HD * h, HD), ds(PB * j, PB)],
                                    qt[p][ds(HD * h, HD), ds(QB * qb + avlo, QB - avlo)],
                                    start=True,
                                    stop=True,
                                )
                            if pending is not None:
                                grp_tail(*pending)
                                pending = None
                            nc.scalar.activation(
                                pg[:, 0:len(grp), ds(avg, QB - avg)],
                                sg[:, 0:len(grp), ds(avg, QB - avg)],
                                mybir.ActivationFunctionType.Exp,
                                scale=1.0 / float(np.sqrt(HD)),
                            )
                            pending = (grp, pg, oacc, qb, p, n_of, total)
                            npairs += 1
                            # staged deferral: the reciprocal/broadcast for
                            # the previous q-block a couple of pairs in, the
                            # normalize multiplies several pairs later (the
                            # GPSIMD broadcast chain is ~6us), the out-proj
                            # matmuls a couple of pairs into the NEXT (p=1)
                            # group.  The last q-block's p=0 half
                            # normalizes early inside (3, p=1) so only the
                            # p=1 half remains for the epilogue.
                            if p == 0:
                                if npairs == 2 and norm_a_q:
                                    norm_a_q.pop(0)()
                                if npairs == 8 and norm_b_q:
                                    norm_b_q.pop(0)()
                            else:
                                if npairs == 2 and outp_q:
                                    outp_q.pop(0)()
                                if qb == NQB - 1:
                                    if npairs == 4:
                                        make_norm_a(qb, (0,))()
                                    if npairs == 10:
                                        make_norm_b(qb, (0,))()
                        if p == 1 and qb < NQB - 1:
                            norm_a_q.append(make_norm_a(qb))
                            norm_b_q.append(make_norm_b(qb))
                            outp_q.append(make_outproj(qb))
                # epilogue: flush the last group's tail, then normalize the
                # last q-block's p=1 half and emit its out-proj
                if pending is not None:
                    grp_tail(*pending)
                    pending = None
                for q in (norm_a_q, norm_b_q):
                    for fn in q:
                        fn()
                # tail normalize for the last q-block's p=1 half: PE is
                # idle here, so broadcast 1/Z with a ones-matmul into PSUM
                # instead of the (serial, ~1us each) GPSIMD broadcasts
                qb = NQB - 1
                zqb = inv_z(qb, (1,))
                for h in (0, 1):
                    if h == 0:
                        src = zqb[0:1, :]
                    else:
                        zr = zpool.tile([1, QB], BF16, tag="zr", bufs=8,
                                        name="zr")
                        nc.gpsimd.dma_start(zr[:], zqb[1:2, :])
                        src = zr[:]
                    zb_ps = opsum.tile([HD, QB], F32, tag="psb", bufs=2,
                                       name="psb")
                    nc.tensor.matmul(
                        zb_ps[:], ones64[:], src, start=True, stop=True
                    )
                    u = ustage.pop((qb, 1, h))
                    nc.vector.tensor_mul(
                        cc[1][ds(HD * h, HD), ds(QB * qb, QB)],
                        u[0:HD, :],
                        zb_ps[:],
                    )
                for fn in outp_q:
                    fn()
                make_outproj(NQB - 1)()

    nc.compile()
    return nc


_CACHE = {}


def _get_program(mask2d, has_bias):
    kind, plan, genmask = _mask_plan(mask2d)
    if kind == "generic":
        key = ("generic", mask2d.tobytes(), has_bias)
    else:
        key = (kind, has_bias)
    if key not in _CACHE:
        _CACHE[key] = (_build(plan, len(genmask), has_bias), genmask)
    return _CACHE[key]


def _chunk_major(a, pb=PB):
    """[R, C] (R = k*pb) -> [pb, k*C] laid out chunk-major."""
    r, c = a.shape
    k = r // pb
    return np.ascontiguousarray(
        a.reshape(k, pb, c).transpose(1, 0, 2).reshape(pb, k * c)
    )


def _make_in_maps(x, mask2d, Wq, bq, Wk, bk, Wv, bv, Wo, bo, genmask, has_bias):
    tri = np.triu(np.ones((PB, PB), dtype=np.float32))
    id64 = np.concatenate(
        [np.zeros((HD, HD), np.float32), np.eye(HD, dtype=np.float32)], axis=0
    )

    bf = mybir.dt.np(BF16)

    in_maps = []
    for i in range(NCORES):
        b, g = divmod(i, 4)
        xbT = np.ascontiguousarray(x[b].T)
        wkv = np.concatenate(
            [Wk[:, HD * g:HD * (g + 1)], Wv[:, HD * g:HD * (g + 1)]], axis=1
        )
        m = {
            "xt": _chunk_major(xbT).astype(bf),
            "wq": _chunk_major(Wq[:, GD * g:GD * (g + 1)]).astype(bf),
            "wkv": _chunk_major(wkv).astype(bf),
            "wo": _chunk_major(Wo[GD * g:GD * (g + 1), :]).astype(bf),
            "tri": tri.astype(bf),
            "ident": id64.astype(bf),
        }
        if len(genmask):
            m["genmask"] = genmask.astype(bf)
        if has_bias:
            bq_g = bq[GD * g:GD * (g + 1)]
            m["bqp"] = np.ascontiguousarray(bq_g.reshape(2, PB).T).astype(np.float32)
            m["bkvp"] = np.concatenate(
                [bk[HD * g:HD * (g + 1)], bv[HD * g:HD * (g + 1)]]
            ).reshape(PB, 1).astype(np.float32)
        in_maps.append(m)
    return in_maps


def _assemble(results, bo):
    out = np.empty((B, T, D), dtype=np.float32)
    for b in range(B):
        acc = None
        for g in range(4):
            r = results[4 * b + g]["outT"]          # [128, 8*2048]
            partial = (
                r.astype(np.float32).reshape(PB, NDC, T).transpose(1, 0, 2).reshape(D, T)
            )
            acc = partial if acc is None else acc + partial
        out[b] = acc.T
    if bo is not None:
        out += bo
    return out


def run(inputs, trace=False):
    from concourse.bass_utils import run_bass_kernel_spmd

    x = np.asarray(inputs["x"], dtype=np.float32)
    mask2d = np.asarray(inputs["mask"]).reshape(T, T).astype(bool)
    Wq = np.asarray(inputs["Wq"], np.float32)
    bq = np.asarray(inputs["bq"], np.float32)
    Wk = np.asarray(inputs["Wk"], np.float32)
    bk = np.asarray(inputs["bk"], np.float32)
    Wv = np.asarray(inputs["Wv"], np.float32)
    bv = np.asarray(inputs["bv"], np.float32)
    Wo = np.asarray(inputs["Wo"], np.float32)
    bo = np.asarray(inputs["bo"], np.float32)
    has_bias = bool(bq.any() or bk.any() or bv.any())
    nc, genmask = _get_program(mask2d, has_bias)
    in_maps = _make_in_maps(
        x, mask2d, Wq, bq, Wk, bk, Wv, bv, Wo, bo, genmask, has_bias
    )
    res = run_bass_kernel_spmd(
        nc, in_maps, core_ids=list(range(NCORES)), trace=trace
    )
    return _assemble(res.results, bo if bo.any() else None), res


def kernel(**inputs) -> np.ndarray:
    out, _ = run(inputs, trace=False)
    return out


# revision 29
# speedup vs baseline: 1.0646x; 1.0427x over previous
"""Grouped-query attention (B=2, T=2048, D=1024, 16 Q heads / 4 KV heads) on
8 Trainium2 NeuronCores — zero-collective version, v2 (pipelined).

Sharding: core i handles batch b = i//4 and head-group g = i%4 (query heads
4g..4g+3, KV head g).  Everything is computed in "transposed" layout
(features on partitions, tokens on the free axis).  bf16 operands with fp32
PSUM accumulation.

v2 changes vs v1 (both correct; v1 measured 205us on HW):
  * KV/Q projections run contraction-outer so the first matmul only waits
    for the first 512KB x^T chunk instead of the whole 4MB load.
  * The softmax normalization is decoupled from the PE critical path:
    oacc (PSUM) is copied out *unnormalized* right after the O-matmuls
    finish (fast DVE copies), the denominators are collected into a
    [16, 512] tile, inverted in one batched reciprocal_approx_fast call
    per q-block, broadcast on GPSIMD, and multiplied into cc on GPSIMD.
    PSUM banks recycle ~5us earlier per group, which keeps the PE busy
    and the HAM clock-gate warm (v1 lost ~60us to per-group PE stalls +
    the resulting 1.2GHz throttled matmuls).
  * out-proj matmuls for q-block qb are deferred into the middle of the
    (qb+1) attention groups so they never stall the PE on the normalize.
  * engine rebalance: Z-row copies and oacc copies on DVE, normalize
    multiplies on GPSIMD, exp stays on ACT (table preloaded at t=0).

No device collectives: each core writes its partial out^T (contraction over
its own 256 concat dims) and the HOST sums the 4 partials per batch.
"""

import sys

if "/opt/trn_rl_repo" not in sys.path:
    sys.path.insert(0, "/opt/trn_rl_repo")

import numpy as np

import concourse.bass as bass
import concourse.mybir as mybir
import concourse.tile as tile
from concourse import bacc, library_config
from concourse.bass import ds

F32 = mybir.dt.float32
BF16 = mybir.dt.bfloat16

B, T, D = 2, 2048, 1024
H, KVH, HD = 16, 4, 64
G = H // KVH            # 4 query heads per core
GD = G * HD             # 256 query dims per core
NCORES = 8
PB = 128                # partition block (s-chunk size)
QB = 512                # q block (matmul moving free dim)
NQB = T // QB           # 4
NSC = T // PB           # 16
NDC = D // PB           # 8 contraction chunks of D
EXP_GRP = 2             # s-chunk slots per exp() call (2 PSUM banks each)


def _mask_plan(mask2d):
    """Per q-block list of (j, avlo, mode, gix).

    mode 0: fully visible chunk.  mode 1: causal-diagonal chunk (columns
    below avlo are dead, the [avlo, avlo+128) square is multiplied by the
    triangular keep-mask).  mode 2: generic chunk, multiplied by keep-mask
    tile gix.  Chunks whose block is fully masked are dropped.
    """
    causal = np.array_equal(mask2d, np.triu(np.ones((T, T), dtype=bool), k=1))
    plan = []
    gen_tiles = []
    if causal:
        for qb in range(NQB):
            chunks = [(j, 0, 0, -1) for j in range(4 * qb)]
            chunks += [(4 * qb + k, PB * k, 1, -1) for k in range(4)]
            plan.append(chunks)
        kind = "causal"
    elif not mask2d.any():
        plan = [[(j, 0, 0, -1) for j in range(NSC)] for _ in range(NQB)]
        kind = "nomask"
    else:
        for qb in range(NQB):
            chunks = []
            for j in range(NSC):
                sub = mask2d[QB * qb:QB * (qb + 1), PB * j:PB * (j + 1)]
                if sub.all():
                    continue
                if not sub.any():
                    chunks.append((j, 0, 0, -1))
                else:
                    gen_tiles.append(np.ascontiguousarray((~sub).T))
                    chunks.append((j, 0, 2, len(gen_tiles) - 1))
            plan.append(chunks)
        kind = "generic"
    genmask = (
        np.stack(gen_tiles, axis=0).astype(np.float32)
        if gen_tiles else np.zeros((0, PB, QB), np.float32)
    )
    return kind, plan, genmask


def _build(plan, ngen, has_bias):
    nc = bacc.Bacc(
        "TRN2", target_bir_lowering=False, debug=False, num_devices=NCORES
    )

    xt_d = nc.dram_tensor("xt", [PB, NDC * T], BF16, kind="ExternalInput")
    wq_d = nc.dram_tensor("wq", [PB, NDC * GD], BF16, kind="ExternalInput")
    wkv_d = nc.dram_tensor("wkv", [PB, NDC * PB], BF16, kind="ExternalInput")
    wo_d = nc.dram_tensor("wo", [PB, 2 * D], BF16, kind="ExternalInput")
    tri_d = nc.dram_tensor("tri", [PB, PB], BF16, kind="ExternalInput")
    id_d = nc.dram_tensor("ident", [PB, HD], BF16, kind="ExternalInput")
    out_d = nc.dram_tensor("outT", [PB, NDC * T], BF16, kind="ExternalOutput")
    gen_d = None
    if ngen:
        gen_d = nc.dram_tensor("genmask", [ngen, PB, QB], BF16, kind="ExternalInput")
    if has_bias:
        bq_d = nc.dram_tensor("bqp", [PB, 2], F32, kind="ExternalInput")
        bkv_d = nc.dram_tensor("bkvp", [PB, 1], F32, kind="ExternalInput")

    NG = 2 * NQB            # number of attention groups (qb, p)

    with tile.TileContext(nc) as tc:
        with (
            tc.tile_pool(name="wts", bufs=1) as wpool,
            tc.tile_pool(name="qkv", bufs=1) as qkvpool,
            tc.tile_pool(name="pp", bufs=4) as ppool,
            tc.tile_pool(name="oo", bufs=2) as opool,
            tc.tile_pool(name="zz", bufs=4) as zpool,
        ):
            # ---- constant loads + ACT exp-table preload ----------------
            tri = wpool.tile([PB, PB], BF16, tag="tri", name="tri")
            nc.sync.dma_start(tri[:], tri_d[:])
            ident = wpool.tile([PB, HD], BF16, tag="ident", name="ident")
            nc.sync.dma_start(ident[:], id_d[:])
            ones64 = wpool.tile([1, HD], BF16, tag="ones64", name="ones64")
            nc.vector.memset(ones64[:], 1.0)
            nc.gpsimd.load_library(library_config.attnmlp)
            warm = wpool.tile([1, 4], F32, tag="warm", name="warm")
            nc.vector.memset(warm[:, 0:2], 0.0)
            nc.scalar.activation(
                warm[:, 2:4], warm[:, 0:2],
                mybir.ActivationFunctionType.Exp, scale=1.0,
            )

            # all inputs on ONE ring in need-order (the HWDGE engines are
            # shared between rings, so two active rings just interleave —
            # a single prioritized stream gets the first-needed tensors
            # to SBUF soonest): wkv, then x^T chunk by chunk, then wq,
            # then the rest.
            wkv_sb = wpool.tile([PB, NDC * PB], BF16, tag="wkv", name="wkv")
            nc.sync.dma_start(wkv_sb[:, 0:PB], wkv_d[:, 0:PB])
            xt = wpool.tile([PB, NDC * T], BF16, tag="xt", name="xt")
            # chunk 0 lands in quarters so the very first matmul (which
            # reads only [:, 0:512]) starts ~1.3us earlier
            for k4 in range(4):
                nc.sync.dma_start(
                    xt[:, ds(QB * k4, QB)], xt_d[:, ds(QB * k4, QB)]
                )
            nc.sync.dma_start(wkv_sb[:, PB:], wkv_d[:, PB:])
            for c in range(1, NDC):
                nc.sync.dma_start(xt[:, ds(T * c, T)], xt_d[:, ds(T * c, T)])
            wq_sb = wpool.tile([PB, NDC * GD], BF16, tag="wq", name="wq")
            for c in range(NDC):
                nc.sync.dma_start(wq_sb[:, ds(GD * c, GD)], wq_d[:, ds(GD * c, GD)])
            wo_sb = wpool.tile([PB, 2 * D], BF16, tag="wo", name="wo")
            nc.sync.dma_start(wo_sb[:], wo_d[:])
            if has_bias:
                bq_sb = wpool.tile([PB, 2], F32, tag="bq", name="bq")
                nc.sync.dma_start(bq_sb[:], bq_d[:])
                bkv_sb = wpool.tile([PB, 1], F32, tag="bkv", name="bkv")
                nc.sync.dma_start(bkv_sb[:], bkv_d[:])

            def psum_to_sbuf(dst, src, bias_col):
                if bias_col is not None:
                    nc.vector.tensor_scalar(
                        dst, src, scalar1=bias_col, scalar2=None,
                        op0=mybir.AluOpType.add,
                    )
                else:
                    nc.vector.tensor_copy(dst, src)

            # ---- projections (contraction-outer: first matmuls start as
            # ---- soon as x^T chunk 0 lands) ----------------------------
            qt = [
                qkvpool.tile([PB, T], BF16, tag="qt0", name="qt0"),
                qkvpool.tile([PB, T], BF16, tag="qt1", name="qt1"),
            ]
            kvt = qkvpool.tile([PB, T], BF16, tag="kvt", name="kvt")
            vt = qkvpool.tile([PB, NSC, HD + 1], BF16, tag="vt", name="vt")
            nc.vector.memset(vt[:, :, HD:HD + 1], 1.0)

            with tc.tile_pool(name="pps", bufs=1, space="PSUM") as pps:
                # KV projection, contraction-outer over 4 PSUM banks
                kv_ps = [
                    pps.tile([PB, QB], F32, tag="qps", bufs=4, name="kvps")
                    for _ in range(NQB)
                ]
                for c in range(NDC):
                    for nqi in range(NQB):
                        nc.tensor.matmul(
                            kv_ps[nqi][:],
                            wkv_sb[:, ds(PB * c, PB)],
                            xt[:, ds(T * c + QB * nqi, QB)],
                            start=(c == 0),
                            stop=(c == NDC - 1),
                        )
                for nqi in range(NQB):
                    psum_to_sbuf(
                        kvt[:, ds(QB * nqi, QB)], kv_ps[nqi][:],
                        bkv_sb[:, 0:1] if has_bias else None,
                    )

                # Q projection p=0 over 4 banks, then p=1 (reuses banks)
                for p in (0, 1):
                    q_ps = [
                        pps.tile([PB, QB], F32, tag="qps", bufs=4, name="qps")
                        for _ in range(NQB)
                    ]
                    for c in range(NDC):
                        for nqi in range(NQB):
                            nc.tensor.matmul(
                                q_ps[nqi][:],
                                wq_sb[:, ds(GD * c + PB * p, PB)],
                                xt[:, ds(T * c + QB * nqi, QB)],
                                start=(c == 0),
                                stop=(c == NDC - 1),
                            )
                    for nqi in range(NQB):
                        psum_to_sbuf(
                            qt[p][:, ds(QB * nqi, QB)], q_ps[nqi][:],
                            bq_sb[:, ds(p, 1)] if has_bias else None,
                        )
                    if p == 0:
                        # V natural tiles (PE transposes of V^T), then
                        # duplicate K^T into rows 64:128 of each block so
                        # head-pair S-matmuls read aligned partition
                        # ranges.  Issued here so the transposes sit on
                        # the PE queue behind Q p=0 (deps long ready).
                        for nqi in range(NQB):
                            v_ps = pps.tile(
                                [PB, 4, HD], BF16, tag="vps", bufs=2, name="vps"
                            )
                            for k4 in range(4):
                                j = 4 * nqi + k4
                                nc.tensor.transpose(
                                    v_ps[:, k4, :],
                                    kvt[HD:PB, ds(PB * j, PB)],
                                    ident[HD:PB, :],
                                )
                            nc.vector.tensor_copy(
                                vt[:, ds(4 * nqi, 4), 0:HD], v_ps[:]
                            )
                            nc.gpsimd.dma_start(
                                kvt[HD:PB, ds(QB * nqi, QB)],
                                kvt[0:HD, ds(QB * nqi, QB)],
                            )

            # ---- attention + interleaved out-proj ----------------------
            cc = [
                qkvpool.tile([PB, T], BF16, tag="cc0", name="cc0"),
                qkvpool.tile([PB, T], BF16, tag="cc1", name="cc1"),
            ]
            # unnormalized O^T + Z staging per (qb, p, h): row 0:64 = O^T,
            # row 64 = Z, in bf16 so the normalize multiplies run in the
            # DVE 4x mode.
            ustage = {}
            zqs = {}

            with (
                tc.tile_pool(name="spsum", bufs=2, space="PSUM") as spsum,
                tc.tile_pool(name="opsum", bufs=1, space="PSUM") as opsum,
            ):
                # deferred-emission event queue keyed by global pair index
                events = {}
                zbmap = {}

                def sched(trig, fn):
                    events.setdefault(trig, []).append(fn)

                def make_outproj(qb):
                    def emit():
                        for pt in range(NDC):
                            o_ps = opsum.tile(
                                [PB, QB], F32, tag="psb", bufs=2, name="psb"
                            )
                            for c2 in (0, 1):
                                nc.tensor.matmul(
                                    o_ps[:],
                                    wo_sb[:, ds(D * c2 + PB * pt, PB)],
                                    cc[c2][:, ds(QB * qb, QB)],
                                    start=(c2 == 0),
                                    stop=(c2 == 1),
                                )
                            ob = opool.tile(
                                [PB, QB], BF16, tag="ob", bufs=8, name="ob"
                            )
                            nc.vector.tensor_copy(ob[:], o_ps[:])
                            nc.sync.dma_start(
                                out_d[:, ds(T * pt + QB * qb, QB)], ob[:]
                            )
                    return emit

                def sched_normalize(G, qb, ps):
                    """Schedule the whole normalize chain for (qb, ps)
                    relative to global pair index G: the reciprocal split
                    into 4 free-dim chunks (each short enough that the
                    masks queued behind it on the DVE barely wait), then
                    bf16 cast + scatter + GPSIMD broadcasts, then the
                    normalize multiplies."""
                    nj = 2 * len(ps)
                    cell = {}

                    def recip_chunk(k):
                        def f():
                            if k == 0:
                                cell["zqi"] = zpool.tile(
                                    [4, QB], F32, tag="zqi", bufs=2,
                                    name="zqi"
                                )
                            nc.vector.reciprocal(
                                cell["zqi"][ds(0, nj), ds(PB * k, PB)],
                                zqs[(qb, ps)][ds(0, nj), ds(PB * k, PB)],
                            )
                        return f

                    def bcasts():
                        zqb = zpool.tile([4, QB], BF16, tag="zqb", bufs=2,
                                         name="zqb")
                        nc.vector.tensor_copy(
                            zqb[ds(0, nj), :], cell["zqi"][ds(0, nj), :]
                        )
                        for p in ps:
                            for h in (0, 1):
                                j = 2 * (p - ps[0]) + h
                                if j == 0:
                                    src = zqb[0:1, :]
                                else:
                                    zr = zpool.tile([1, QB], BF16, tag="zr",
                                                    bufs=8, name="zr")
                                    nc.gpsimd.dma_start(zr[:], zqb[j:j + 1, :])
                                    src = zr[:]
                                zb = zpool.tile([HD, QB], BF16, tag="zb",
                                                bufs=8, name="zb")
                                nc.gpsimd.partition_broadcast(zb[:], src)
                                zbmap[(qb, p, h)] = zb

                    def muls():
                        for p in ps:
                            for h in (0, 1):
                                u = ustage.pop((qb, p, h))
                                zb = zbmap.pop((qb, p, h))
                                nc.vector.tensor_mul(
                                    cc[p][ds(HD * h, HD), ds(QB * qb, QB)],
                                    u[0:HD, :],
                                    zb[:],
                                )

                    for k in range(4):
                        sched(G + 2 + 2 * k, recip_chunk(k))
                    sched(G + 10, bcasts)
                    sched(G + 12, muls)

                pending = None     # (grp, pg, oacc, qb, p, state) for grp_tail

                def grp_tail(grp, pg, oacc, qb, p, n_of, total):
                    # tri/generic masking (DVE) + O-matmuls (PE); when a
                    # head's accumulation completes, evacuate the whole
                    # [65, QB] PSUM tile (O^T + Z row) to SBUF in one fast
                    # copy so the bank recycles immediately.
                    for idx, (h, (j, avlo, mode, gix)) in enumerate(grp):
                        if mode == 1:
                            nc.vector.tensor_mul(
                                pg[:, idx, ds(avlo, PB)],
                                pg[:, idx, ds(avlo, PB)],
                                tri[:],
                            )
                        elif mode == 2:
                            gm = ppool.tile(
                                [PB, QB], BF16, tag="gm", bufs=4, name="gm"
                            )
                            nc.sync.dma_start(gm[:], gen_d[gix])
                            nc.vector.tensor_mul(
                                pg[:, idx, :], pg[:, idx, :], gm[:]
                            )
                        nc.tensor.matmul(
                            oacc[h][:, ds(avlo, QB - avlo)],
                            vt[:, j, :],
                            pg[:, idx, ds(avlo, QB - avlo)],
                            start=(n_of[h] == 0),
                            stop=(n_of[h] == total - 1),
                        )
                        n_of[h] += 1
                        if n_of[h] == total:
                            u = zpool.tile([HD + 1, QB], BF16, tag="ustg",
                                           bufs=6, name="ustg")
                            nc.vector.tensor_copy(u[:], oacc[h][:])
                            ustage[(qb, p, h)] = u
                            # eager Z-row gather on the (idle) GPSIMD ring;
                            # the last q-block normalizes per-p so it
                            # gathers into per-p tiles (rows must start at
                            # partition 0)
                            ps = ((p,) if qb == NQB - 1 else (0, 1))
                            if (qb, ps) not in zqs:
                                zqs[(qb, ps)] = zpool.tile(
                                    [4, QB], BF16, tag="zq", bufs=3, name="zq"
                                )
                            j = 2 * (p - ps[0]) + h
                            nc.gpsimd.dma_start(
                                zqs[(qb, ps)][j:j + 1, :],
                                u[HD:HD + 1, :],
                            )

                gpi = 0     # global pair index across all groups
                for qb in range(NQB):
                    chunks = plan[qb]
                    for p in (0, 1):
                        slots = [(h, ch) for ch in chunks for h in (0, 1)]
                        oacc = [
                            opsum.tile([HD + 1, QB], F32, tag="oacc", bufs=2,
                                       name="oacc")
                            for _ in (0, 1)
                        ]
                        n_of = {0: 0, 1: 0}
                        total = len(chunks)
                        npairs = 0
                        for gi0 in range(0, len(slots), EXP_GRP):
                            grp = slots[gi0:gi0 + EXP_GRP]
                            avg = grp[0][1][1]
                            sg = spsum.tile(
                                [PB, EXP_GRP, QB], F32, tag="sg", bufs=2,
                                name="sg"
                            )
                            pg = ppool.tile(
                                [PB, EXP_GRP, QB], BF16, tag="pg", bufs=6,
                                name="pg"
                            )
                            for idx, (h, (j, avlo, mode, gix)) in enumerate(grp):
                                nc.tensor.matmul(
                                    sg[:, idx, ds(avlo, QB - avlo)],
                                    kvt[ds(HD * h, HD), ds(PB * j, PB)],
                                    qt[p][ds(HD * h, HD), ds(QB * qb + avlo, QB - avlo)],
                                    start=True,
                                    stop=True,
                                )
                            if pending is not None:
                                grp_tail(*pending)
                                pending = None
                            nc.scalar.activation(
                                pg[:, 0:len(grp), ds(avg, QB - avg)],
                                sg[:, 0:len(grp), ds(avg, QB - avg)],
                                mybir.ActivationFunctionType.Exp,
                                scale=1.0 / float(np.sqrt(HD)),
                            )
                            pending = (grp, pg, oacc, qb, p, n_of, total)
                            npairs += 1
                            gpi += 1
                            for fn in events.pop(gpi, []):
                                fn()
                        if p == 1 and qb < NQB - 1:
                            sched_normalize(gpi, qb, (0, 1))
                            sched(gpi + 16, make_outproj(qb))
                        if p == 0 and qb == NQB - 1:
                            sched_normalize(gpi, qb, (0,))
                # epilogue: flush the last group's tail and any remaining
                # scheduled work, then normalize the last q-block's p=1
                # half (PE is idle here, so broadcast 1/Z with a
                # ones-matmul into PSUM instead of the serial GPSIMD
                # broadcasts) and emit its out-proj
                if pending is not None:
                    grp_tail(*pending)
                    pending = None
                for trig in sorted(events):
                    for fn in events.pop(trig, []):
                        fn()
                qb = NQB - 1
                zq3 = zqs[(qb, (1,))]
                zqi3 = zpool.tile([4, QB], F32, tag="zqi", bufs=2, name="zqi")
                nc.vector.reciprocal(zqi3[0:2, :], zq3[0:2, :])
                zqb3 = zpool.tile([4, QB], BF16, tag="zqb", bufs=2, name="zqb")
                nc.vector.tensor_copy(zqb3[0:2, :], zqi3[0:2, :])
                for h in (0, 1):
                    if h == 0:
                        src = zqb3[0:1, :]
                    else:
                        zr = zpool.tile([1, QB], BF16, tag="zr", bufs=8,
                                        name="zr")
                        nc.gpsimd.dma_start(zr[:], zqb3[1:2, :])
                        src = zr[:]
                    zb_ps = opsum.tile([HD, QB], F32, tag="psb", bufs=2,
                                       name="psb")
                    nc.tensor.matmul(
                        zb_ps[:], ones64[:], src, start=True, stop=True
                    )
                    u = ustage.pop((qb, 1, h))
                    nc.vector.tensor_mul(
                        cc[1][ds(HD * h, HD), ds(QB * qb, QB)],
                        u[0:HD, :],
                        zb_ps[:],
                    )
                make_outproj(NQB - 1)()

    nc.compile()
    return nc


_CACHE = {}


def _get_program(mask2d, has_bias):
    kind, plan, genmask = _mask_plan(mask2d)
    if kind == "generic":
        key = ("generic", mask2d.tobytes(), has_bias)
    else:
        key = (kind, has_bias)
    if key not in _CACHE:
        _CACHE[key] = (_build(plan, len(genmask), has_bias), genmask)
    return _CACHE[key]


def _chunk_major(a, pb=PB):
    """[R, C] (R = k*pb) -> [pb, k*C] laid out chunk-major."""
    r, c = a.shape
    k = r // pb
    return np.ascontiguousarray(
        a.reshape(k, pb, c).transpose(1, 0, 2).reshape(pb, k * c)
    )


def _make_in_maps(x, mask2d, Wq, bq, Wk, bk, Wv, bv, Wo, bo, genmask, has_bias):
    tri = np.triu(np.ones((PB, PB), dtype=np.float32))
    id64 = np.concatenate(
        [np.zeros((HD, HD), np.float32), np.eye(HD, dtype=np.float32)], axis=0
    )

    bf = mybir.dt.np(BF16)

    in_maps = []
    for i in range(NCORES):
        b, g = divmod(i, 4)
        xbT = np.ascontiguousarray(x[b].T)
        wkv = np.concatenate(
            [Wk[:, HD * g:HD * (g + 1)], Wv[:, HD * g:HD * (g + 1)]], axis=1
        )
        m = {
            "xt": _chunk_major(xbT).astype(bf),
            "wq": _chunk_major(Wq[:, GD * g:GD * (g + 1)]).astype(bf),
            "wkv": _chunk_major(wkv).astype(bf),
            "wo": _chunk_major(Wo[GD * g:GD * (g + 1), :]).astype(bf),
            "tri": tri.astype(bf),
            "ident": id64.astype(bf),
        }
        if len(genmask):
            m["genmask"] = genmask.astype(bf)
        if has_bias:
            bq_g = bq[GD * g:GD * (g + 1)]
            m["bqp"] = np.ascontiguousarray(bq_g.reshape(2, PB).T).astype(np.float32)
            m["bkvp"] = np.concatenate(
                [bk[HD * g:HD * (g + 1)], bv[HD * g:HD * (g + 1)]]
            ).reshape(PB, 1).astype(np.float32)
        in_maps.append(m)
    return in_maps


def _assemble(results, bo):
    out = np.empty((B, T, D), dtype=np.float32)
    for b in range(B):
        acc = None
        for g in range(4):
            r = results[4 * b + g]["outT"]          # [128, 8*2048]
            partial = (
                r.astype(np.float32).reshape(PB, NDC, T).transpose(1, 0, 2).reshape(D, T)
            )
            acc = partial if acc is None else acc + partial
        out[b] = acc.T
    if bo is not None:
        out += bo
    return out


def run(inputs, trace=False):
    from concourse.bass_utils import run_bass_kernel_spmd

    x = np.asarray(inputs["x"], dtype=np.float32)
    mask2d = np.asarray(inputs["mask"]).reshape(T, T).astype(bool)
    Wq = np.asarray(inputs["Wq"], np.float32)
    bq = np.asarray(inputs["bq"], np.float32)
    Wk = np.asarray(inputs["Wk"], np.float32)
    bk = np.asarray(inputs["bk"], np.float32)
    Wv = np.asarray(inputs["Wv"], np.float32)
    bv = np.asarray(inputs["bv"], np.float32)
    Wo = np.asarray(inputs["Wo"], np.float32)
    bo = np.asarray(inputs["bo"], np.float32)
    has_bias = bool(bq.any() or bk.any() or bv.any())
    nc, genmask = _get_program(mask2d, has_bias)
    in_maps = _make_in_maps(
        x, mask2d, Wq, bq, Wk, bk, Wv, bv, Wo, bo, genmask, has_bias
    )
    res = run_bass_kernel_spmd(
        nc, in_maps, core_ids=list(range(NCORES)), trace=trace
    )
    return _assemble(res.results, bo if bo.any() else None), res


def kernel(**inputs) -> np.ndarray:
    out, _ = run(inputs, trace=False)
    return out
